# revision 4
# baseline (speedup 1.0000x reference)
"""Bipartite graph convolution on 8 Trainium2 NeuronCores.

Math (from the reference):
    edge  = (adj > 0) as f32                      [N_u, N_v], values 0/1
    out_u = relu((edge @ v_feat / rowdeg) @ W_u)  [N_u, 32]
    out_v = relu((edge.T @ u_feat / coldeg) @ W_v)[N_v, 32]

Distribution: adj rows sharded across the 8 cores (each core owns 1024 rows).
Each core streams its 64MB int32 shard once (memory-bound side), building a
bf16 0/1 edge matrix on the fly, and computes BOTH matmul orientations:
  - v-side (contract over u): natural layout,  X_v += ufa.T @ edge   (partial)
  - u-side (contract over v): XBAR-transposed, X_u  = vfa.T @ edge.T (complete)
Features are shipped as bf16 [hi | ones | lo] stacked columns (M=65) so one
matmul pass produces hi- and lo-precision partials plus the degree row; the
hi+lo fold restores ~f32 feature precision at no extra PE streaming cost.
Host: gathers X_u, all-reduces X_v partials, folds hi+lo, normalizes by the
degree row, applies the 32x32 weights + relu (~0.1% of total FLOPs).
"""
import sys
if '/opt/trn_rl_repo' not in sys.path:
    sys.path.insert(0, '/opt/trn_rl_repo')

import numpy as np
import ml_dtypes

from concourse import bass, mybir, tile
from concourse.bass_utils import run_bass_kernel_spmd
from concourse.vector_clock import ScopedClock

BF16 = mybir.dt.bfloat16
F32 = mybir.dt.float32

N_U, N_V = 8192, 16384
N_CORES = 8
R = N_U // N_CORES          # 1024 adj rows per core
SLAB = 2048                 # v-columns processed per slab
NSLAB = N_V // SLAB         # 8
NUB = R // 128              # 8 u-panels of 128 rows
NVB = SLAB // 128           # 16 v-blocks per slab
M = 65                      # feature columns: hi32 | ones | lo32


# --- walrus compatibility: this container's walrus rejects >1 sync-wait per
# instruction, but Tile's scheduler can attach several (tail drain, DMAs with
# multiple producers, ...). Hoist extra waits onto single-wait NOPs inserted
# just before the instruction on the same engine stream.
def _split_multi_waits(nc):
    for bb in nc.m.functions[0].blocks:
        il = bb.instructions
        out = []
        changed = False
        for inst in il:
            si = inst.sync_info
            if si is not None and si.on_wait and len(si.on_wait) > 1:
                waits = list(si.on_wait)
                for i, w in enumerate(waits[:-1]):
                    nop = mybir.InstNoOp(name=f"{inst.name}-sw{i}",
                                         ins=[], outs=[])
                    nop.engine = inst.engine
                    nop.sync_info = mybir.SyncInfo(on_wait=[w], on_update=[])
                    nc.register_instruction(nop, overwrite=True)
                    out.append(nop)
                si.on_wait = waits[-1:]
                inst.sync_info = si
                changed = True
            out.append(inst)
        if changed:
            bb.instructions = out


def _emit_body(nc, tc, pools, adj, ufa_t, vfa_t, xu, xv):
    featp, edgep, edgeTp, adjp, stagep, vpsp, upsp = pools
    is_gt = mybir.AluOpType.is_gt

    upsum = upsp.tile([M, R], F32)          # persistent u-side accumulator

    for s in range(NSLAB):
        # load + edge build: adj int32 -> (cast-DMA) bf16 -> (is_gt 0) 0/1
        edge = edgep.tile([128, NUB, SLAB], BF16)
        for ub in range(NUB):
            adjf = adjp.tile([128, SLAB], BF16, tag="adjf")
            nc.gpsimd.dma_start(
                adjf[:], adj[ub * 128:(ub + 1) * 128, s * SLAB:(s + 1) * SLAB])
            nc.vector.tensor_scalar(
                edge[:, ub, :], adjf[:], 0.0, None, op0=is_gt)

        # v-side: X_v[:, slab] = sum_ub ufa[ub].T @ edge[ub]
        for h in range(2):                  # half-slab = 1024 v-cols
            vpsum = vpsp.tile([M, 1024], F32, tag="vpsum")
            for ub in range(NUB):
                for nb in range(2):
                    c0 = h * 1024 + nb * 512
                    nc.tensor.matmul(
                        vpsum[:, nb * 512:(nb + 1) * 512],
                        ufa_t[:, ub, :],
                        edge[:, ub, c0:c0 + 512],
                        start=(ub == 0), stop=(ub == NUB - 1))
            xv_s = stagep.tile([M, 1024], F32, tag="xv_s")
            nc.any.tensor_copy(xv_s[:], vpsum[:])
            nc.sync.dma_start(
                xv[:, s * SLAB + h * 1024: s * SLAB + (h + 1) * 1024], xv_s[:])

        # transpose the slab: edgeT[p, vb, u] = edge[u, vb*128 + p]
        edgeT = edgeTp.tile([128, NVB, R], BF16)
        for ub in range(NUB):
            nc.sync.dma_start(
                out=edgeT[:, :, ub * 128:(ub + 1) * 128],
                in_=edge[:, ub, :],
                transpose=True)

        # u-side: X_u += sum_vb vfa[slab, vb].T @ edgeT[vb]
        for vb in range(NVB):
            for nb in range(R // 512):
                nc.tensor.matmul(
                    upsum[:, nb * 512:(nb + 1) * 512],
                    vfa_t[:, s * NVB + vb, :],
                    edgeT[:, vb, nb * 512:(nb + 1) * 512],
                    start=(s == 0 and vb == 0),
                    stop=(s == NSLAB - 1 and vb == NVB - 1))

    xu_s = stagep.tile([M, R], F32, tag="xu_s")
    nc.any.tensor_copy(xu_s[:], upsum[:])
    nc.sync.dma_start(xu[:], xu_s[:])


def build_nc(repeat: int = 1):
    """One SPMD program: full inputs per core are its row shard of adj plus
    pre-tiled bf16 feature tensors; outputs are the feature-major partials."""
    nc = bass.Bass("TRN2", target_bir_lowering=False, debug=False)
    adj = nc.dram_tensor("adj", [R, N_V], mybir.dt.int32, kind="ExternalInput")
    # pre-tiled on host: [128, ntiles*M] with [p, a*M+m] = feat[a*128+p, m]
    ufa = nc.dram_tensor("ufa", [128, NUB * M], BF16, kind="ExternalInput")
    vfa = nc.dram_tensor("vfa", [128, (N_V // 128) * M], BF16,
                         kind="ExternalInput")
    xu = nc.dram_tensor("xu", [M, R], F32, kind="ExternalOutput")
    xv = nc.dram_tensor("xv", [M, N_V], F32, kind="ExternalOutput")

    with tile.TileContext(nc) as tc:
        with tc.tile_pool(name="feat", bufs=1) as featp, \
             tc.tile_pool(name="edge", bufs=2) as edgep, \
             tc.tile_pool(name="edgeT", bufs=2) as edgeTp, \
             tc.tile_pool(name="adj", bufs=3) as adjp, \
             tc.tile_pool(name="stage", bufs=2) as stagep, \
             tc.tile_pool(name="vps", bufs=2, space="PSUM") as vpsp, \
             tc.tile_pool(name="ups", bufs=1, space="PSUM") as upsp:
            ufa_t = featp.tile([128, NUB, M], BF16)
            nc.sync.dma_start(ufa_t[:], ufa.rearrange("p (a m) -> p a m", m=M))
            vfa_t = featp.tile([128, N_V // 128, M], BF16)
            nc.sync.dma_start(vfa_t[:], vfa.rearrange("p (a m) -> p a m", m=M))
            pools = (featp, edgep, edgeTp, adjp, stagep, vpsp, upsp)
            for _ in range(repeat):
                _emit_body(nc, tc, pools, adj, ufa_t, vfa_t, xu, xv)
    _split_multi_waits(nc)
    return nc


def _tile_features(feat32: np.ndarray) -> np.ndarray:
    """[N,32] f32 -> pre-tiled [128, (N//128)*65] bf16 of [hi32|ones|lo32]."""
    n = feat32.shape[0]
    hi = feat32.astype(ml_dtypes.bfloat16)
    lo = (feat32 - hi.astype(np.float32)).astype(ml_dtypes.bfloat16)
    aug = np.zeros((n, M), dtype=ml_dtypes.bfloat16)
    aug[:, 0:32] = hi
    aug[:, 32] = 1.0
    aug[:, 33:65] = lo
    # [N, M] -> [ntiles, 128, M] -> [128, ntiles, M] -> [128, ntiles*M]
    return np.ascontiguousarray(
        aug.reshape(n // 128, 128, M).transpose(1, 0, 2).reshape(128, -1))


def _finalize(x: np.ndarray, w: np.ndarray) -> np.ndarray:
    """x [65, N] feature-major raw sums -> relu((num/deg) @ w) [N, 32]."""
    num = (x[0:32].astype(np.float32) + x[33:65].astype(np.float32)).T
    deg = x[32]
    agg = num / deg[:, None]
    return np.maximum(agg @ w.astype(np.float32), 0.0).astype(np.float32)


_NC_CACHE: dict = {}


def _get_nc(repeat: int = 1):
    if repeat not in _NC_CACHE:
        _NC_CACHE[repeat] = build_nc(repeat)
    return _NC_CACHE[repeat]


def _device_inputs(adj, u_feature, v_feature):
    adj = np.asarray(adj)
    if adj.dtype != np.int32:
        adj = adj.astype(np.int32)
    ufa_full = _tile_features(np.asarray(u_feature, dtype=np.float32))
    vfa = _tile_features(np.asarray(v_feature, dtype=np.float32))
    in_maps = []
    for c in range(N_CORES):
        in_maps.append({
            "adj": np.ascontiguousarray(adj[c * R:(c + 1) * R]),
            # ufa pre-tiling must be per-shard: tile the shard's rows
            "ufa": None,  # filled below
            "vfa": vfa,
        })
    uf = np.asarray(u_feature, dtype=np.float32)
    for c in range(N_CORES):
        in_maps[c]["ufa"] = _tile_features(uf[c * R:(c + 1) * R])
    return in_maps


def kernel(adj, u_feature, v_feature, weight_u, weight_v, _repeat: int = 1):
    in_maps = _device_inputs(adj, u_feature, v_feature)
    nc = _get_nc(_repeat)
    res = run_bass_kernel_spmd(nc, in_maps, core_ids=list(range(N_CORES)))
    xu = np.concatenate([res.results[c]["xu"] for c in range(N_CORES)], axis=1)
    xv = np.sum([res.results[c]["xv"] for c in range(N_CORES)], axis=0)
    w_u = np.asarray(weight_u, dtype=np.float32)
    w_v = np.asarray(weight_v, dtype=np.float32)
    output_u = _finalize(xu, w_u)
    output_v = _finalize(xv, w_v)
    return output_u, output_v


# revision 12
# speedup vs baseline: 270.2839x; 270.2839x over previous
"""Bipartite graph convolution on 8 Trainium2 NeuronCores.

Math (from the reference):
    edge  = (adj > 0) as f32                      [N_u, N_v], values 0/1
    out_u = relu((edge @ v_feat / rowdeg) @ W_u)  [N_u, 32]
    out_v = relu((edge.T @ u_feat / coldeg) @ W_v)[N_v, 32]

Distribution: adj rows sharded across the 8 cores (each core owns 1024 rows).
Each core streams its 64MB int32 shard once (memory-bound side), building a
bf16 0/1 edge matrix on the fly, and computes BOTH matmul orientations:
  - v-side (contract over u): natural layout,  X_v += ufa.T @ edge   (partial)
  - u-side (contract over v): XBAR-transposed, X_u  = vfa.T @ edge.T (complete)
Features are shipped as bf16 [hi | ones | lo] stacked columns (M=65) so one
matmul pass produces hi- and lo-precision partials plus the degree row; the
hi+lo fold restores ~f32 feature precision at no extra PE streaming cost.
Host: gathers X_u, all-reduces X_v partials, folds hi+lo, normalizes by the
degree row, applies the 32x32 weights + relu (~0.1% of total FLOPs).
"""
import sys
if '/opt/trn_rl_repo' not in sys.path:
    sys.path.insert(0, '/opt/trn_rl_repo')

import numpy as np
import ml_dtypes

from concourse import bass, mybir, tile
from concourse.bass_utils import run_bass_kernel_spmd
from concourse.vector_clock import ScopedClock

BF16 = mybir.dt.bfloat16
F32 = mybir.dt.float32

N_U, N_V = 8192, 16384
N_CORES = 8
R = N_U // N_CORES          # 1024 adj rows per core
SLAB = 2048                 # v-columns processed per slab
NSLAB = N_V // SLAB         # 8
NUB = R // 128              # 8 u-panels of 128 rows
NVB = SLAB // 128           # 16 v-blocks per slab
M = 65                      # feature columns: hi32 | ones | lo32
PE_SLABS = 8                # slabs transposed on the PE instead of DMA XBAR
                            # (XBAR transposes serialize against other DMAs on
                            # this HW; PE-transpose measured ~3.6x faster)


# --- walrus compatibility: this container's walrus rejects >1 sync-wait per
# instruction, but Tile's scheduler can attach several (tail drain, DMAs with
# multiple producers, ...). Hoist extra waits onto single-wait NOPs inserted
# just before the instruction on the same engine stream.
def _split_multi_waits(nc):
    for bb in nc.m.functions[0].blocks:
        il = bb.instructions
        out = []
        changed = False
        for inst in il:
            si = inst.sync_info
            if si is not None and si.on_wait and len(si.on_wait) > 1:
                waits = list(si.on_wait)
                for i, w in enumerate(waits[:-1]):
                    nop = mybir.InstNoOp(name=f"{inst.name}-sw{i}",
                                         ins=[], outs=[])
                    nop.engine = inst.engine
                    nop.sync_info = mybir.SyncInfo(on_wait=[w], on_update=[])
                    nc.register_instruction(nop, overwrite=True)
                    out.append(nop)
                si.on_wait = waits[-1:]
                inst.sync_info = si
                changed = True
            out.append(inst)
        if changed:
            bb.instructions = out


def _emit_body(nc, tc, pools, adj, ufa_t, vfa_t, xu, xv, ident_s=None,
               pe_slabs=0):
    featp, edgep, edgeTp, adjp, stagep, vpsp, upsp, tpp = pools
    is_gt = mybir.AluOpType.is_gt

    upsum = upsp.tile([M, R], F32)          # persistent u-side accumulator

    for s in range(NSLAB):
        # load + edge build: adj int32 -> (cast-DMA) bf16 -> (is_gt 0) 0/1
        edge = edgep.tile([128, NUB, SLAB], BF16)
        for ub in range(NUB):
            adjf = adjp.tile([128, SLAB], BF16, tag="adjf")
            nc.gpsimd.dma_start(
                adjf[:], adj[ub * 128:(ub + 1) * 128, s * SLAB:(s + 1) * SLAB])
            nc.vector.tensor_scalar(
                edge[:, ub, :], adjf[:], 0.0, None, op0=is_gt)

        # v-side: X_v[:, slab] = sum_ub ufa[ub].T @ edge[ub]
        for h in range(2):                  # half-slab = 1024 v-cols
            vpsum = vpsp.tile([M, 1024], F32, tag="vpsum")
            for ub in range(NUB):
                for nb in range(2):
                    c0 = h * 1024 + nb * 512
                    nc.tensor.matmul(
                        vpsum[:, nb * 512:(nb + 1) * 512],
                        ufa_t[:, ub, :],
                        edge[:, ub, c0:c0 + 512],
                        start=(ub == 0), stop=(ub == NUB - 1))
            xv_s = stagep.tile([M, 1024], F32, tag="xv_s")
            nc.any.tensor_copy(xv_s[:], vpsum[:])
            nc.sync.dma_start(
                xv[:, s * SLAB + h * 1024: s * SLAB + (h + 1) * 1024], xv_s[:])

        # transpose the slab: edgeT[p, vb, u] = edge[u, vb*128 + p]
        # First `pe_slabs` slabs transpose on the PE (via identity matmul +
        # ACT copy-out), the rest on the DMA XBAR — balances the SDMA pool
        # against the tensor engine.
        edgeT = edgeTp.tile([128, NVB, R], BF16)
        if s < pe_slabs:
            for ub in range(NUB):
                for h in range(2):          # 1024-col chunk = 1 PSUM bank
                    tp = tpp.tile([128, NVB // 2, 128], BF16, tag="tp")
                    for vb in range(NVB // 2):
                        nc.tensor.transpose(
                            tp[:, vb, :],
                            edge[:, ub, h * 1024 + vb * 128:
                                 h * 1024 + (vb + 1) * 128],
                            ident_s[:])
                    nc.scalar.activation(
                        edgeT[:, h * (NVB // 2):(h + 1) * (NVB // 2),
                              ub * 128:(ub + 1) * 128],
                        tp[:],
                        mybir.ActivationFunctionType.Copy)
        else:
            for ub in range(NUB):
                nc.sync.dma_start(
                    out=edgeT[:, :, ub * 128:(ub + 1) * 128],
                    in_=edge[:, ub, :],
                    transpose=True)

        # u-side: X_u += sum_vb vfa[slab, vb].T @ edgeT[vb]
        for vb in range(NVB):
            for nb in range(R // 512):
                nc.tensor.matmul(
                    upsum[:, nb * 512:(nb + 1) * 512],
                    vfa_t[:, s * NVB + vb, :],
                    edgeT[:, vb, nb * 512:(nb + 1) * 512],
                    start=(s == 0 and vb == 0),
                    stop=(s == NSLAB - 1 and vb == NVB - 1))

    xu_s = stagep.tile([M, R], F32, tag="xu_s")
    nc.any.tensor_copy(xu_s[:], upsum[:])
    nc.sync.dma_start(xu[:], xu_s[:])


def build_nc(repeat: int = 1, pe_slabs: int = PE_SLABS):
    """One SPMD program: full inputs per core are its row shard of adj plus
    pre-tiled bf16 feature tensors; outputs are the feature-major partials."""
    nc = bass.Bass("TRN2", target_bir_lowering=False, debug=False)
    adj = nc.dram_tensor("adj", [R, N_V], mybir.dt.int32, kind="ExternalInput")
    # pre-tiled on host: [128, ntiles*M] with [p, a*M+m] = feat[a*128+p, m]
    ufa = nc.dram_tensor("ufa", [128, NUB * M], BF16, kind="ExternalInput")
    vfa = nc.dram_tensor("vfa", [128, (N_V // 128) * M], BF16,
                         kind="ExternalInput")
    ident = nc.dram_tensor("ident", [128, 128], BF16, kind="ExternalInput")
    xu = nc.dram_tensor("xu", [M, R], F32, kind="ExternalOutput")
    xv = nc.dram_tensor("xv", [M, N_V], F32, kind="ExternalOutput")

    with tile.TileContext(nc) as tc:
        with tc.tile_pool(name="feat", bufs=1) as featp, \
             tc.tile_pool(name="edge", bufs=2) as edgep, \
             tc.tile_pool(name="edgeT", bufs=2) as edgeTp, \
             tc.tile_pool(name="adj", bufs=3) as adjp, \
             tc.tile_pool(name="stage", bufs=2) as stagep, \
             tc.tile_pool(name="vps", bufs=2, space="PSUM") as vpsp, \
             tc.tile_pool(name="ups", bufs=1, space="PSUM") as upsp, \
             tc.tile_pool(name="tp", bufs=2, space="PSUM") as tpp:
            ufa_t = featp.tile([128, NUB, M], BF16)
            nc.sync.dma_start(ufa_t[:], ufa.rearrange("p (a m) -> p a m", m=M))
            vfa_t = featp.tile([128, N_V // 128, M], BF16)
            nc.sync.dma_start(vfa_t[:], vfa.rearrange("p (a m) -> p a m", m=M))
            ident_s = featp.tile([128, 128], BF16)
            nc.sync.dma_start(ident_s[:], ident[:])
            pools = (featp, edgep, edgeTp, adjp, stagep, vpsp, upsp, tpp)
            for _ in range(repeat):
                _emit_body(nc, tc, pools, adj, ufa_t, vfa_t, xu, xv,
                           ident_s=ident_s, pe_slabs=pe_slabs)
    _split_multi_waits(nc)
    return nc


def _tile_features(feat32: np.ndarray) -> np.ndarray:
    """[N,32] f32 -> pre-tiled [128, (N//128)*65] bf16 of [hi32|ones|lo32]."""
    n = feat32.shape[0]
    hi = feat32.astype(ml_dtypes.bfloat16)
    lo = (feat32 - hi.astype(np.float32)).astype(ml_dtypes.bfloat16)
    aug = np.zeros((n, M), dtype=ml_dtypes.bfloat16)
    aug[:, 0:32] = hi
    aug[:, 32] = 1.0
    aug[:, 33:65] = lo
    # [N, M] -> [ntiles, 128, M] -> [128, ntiles, M] -> [128, ntiles*M]
    return np.ascontiguousarray(
        aug.reshape(n // 128, 128, M).transpose(1, 0, 2).reshape(128, -1))


def _finalize(x: np.ndarray, w: np.ndarray) -> np.ndarray:
    """x [65, N] feature-major raw sums -> relu((num/deg) @ w) [N, 32]."""
    num = (x[0:32].astype(np.float32) + x[33:65].astype(np.float32)).T
    deg = x[32]
    agg = num / deg[:, None]
    return np.maximum(agg @ w.astype(np.float32), 0.0).astype(np.float32)


_NC_CACHE: dict = {}


def _get_nc(repeat: int = 1, pe_slabs: int = PE_SLABS):
    key = (repeat, pe_slabs)
    if key not in _NC_CACHE:
        _NC_CACHE[key] = build_nc(repeat, pe_slabs)
    return _NC_CACHE[key]


def _device_inputs(adj, u_feature, v_feature):
    adj = np.asarray(adj)
    if adj.dtype != np.int32:
        adj = adj.astype(np.int32)
    ufa_full = _tile_features(np.asarray(u_feature, dtype=np.float32))
    vfa = _tile_features(np.asarray(v_feature, dtype=np.float32))
    ident = np.eye(128, dtype=ml_dtypes.bfloat16)
    uf = np.asarray(u_feature, dtype=np.float32)
    in_maps = []
    for c in range(N_CORES):
        in_maps.append({
            "adj": np.ascontiguousarray(adj[c * R:(c + 1) * R]),
            "ufa": _tile_features(uf[c * R:(c + 1) * R]),
            "vfa": vfa,
            "ident": ident,
        })
    return in_maps


def kernel(adj, u_feature, v_feature, weight_u, weight_v, _repeat: int = 1):
    in_maps = _device_inputs(adj, u_feature, v_feature)
    nc = _get_nc(_repeat)
    res = run_bass_kernel_spmd(nc, in_maps, core_ids=list(range(N_CORES)))
    xu = np.concatenate([res.results[c]["xu"] for c in range(N_CORES)], axis=1)
    xv = np.sum([res.results[c]["xv"] for c in range(N_CORES)], axis=0)
    w_u = np.asarray(weight_u, dtype=np.float32)
    w_v = np.asarray(weight_v, dtype=np.float32)
    output_u = _finalize(xu, w_u)
    output_v = _finalize(xv, w_v)
    return output_u, output_v


# revision 17
# speedup vs baseline: 7330.4316x; 27.1212x over previous
"""Bipartite graph convolution on 8 Trainium2 NeuronCores.

Math (from the reference):
    edge  = (adj > 0) as f32                      [N_u, N_v], values 0/1
    out_u = relu((edge @ v_feat / rowdeg) @ W_u)  [N_u, 32]
    out_v = relu((edge.T @ u_feat / coldeg) @ W_v)[N_v, 32]

Distribution: adj rows sharded across the 8 cores (each core owns 1024 rows).
Each core streams its 64MB int32 shard once (memory-bound side), building a
bf16 0/1 edge matrix on the fly, and computes BOTH matmul orientations:
  - v-side (contract over u): natural layout, X_v += ufa.T @ edge   (partial)
  - u-side (contract over v): PE-transposed,  X_u  = vfa.T @ edge.T (complete)
Features are shipped as bf16 [hi | ones | lo] stacked columns (M=65) so one
matmul pass produces hi- and lo-precision partials plus the degree row; the
hi+lo fold restores ~f32 feature precision at no extra PE streaming cost.
Host: gathers X_u, all-reduces X_v partials, folds hi+lo, normalizes by the
degree row, applies the 32x32 weights + relu (~0.1% of total FLOPs).
"""
import sys
if '/opt/trn_rl_repo' not in sys.path:
    sys.path.insert(0, '/opt/trn_rl_repo')

import numpy as np
import ml_dtypes

from concourse import bass, mybir, tile
from concourse.bass_utils import run_bass_kernel_spmd

BF16 = mybir.dt.bfloat16
F32 = mybir.dt.float32

N_U, N_V = 8192, 16384
N_CORES = 8
R = N_U // N_CORES          # 1024 adj rows per core
SLAB = 2048                 # v-columns processed per slab
NSLAB = N_V // SLAB         # 8
NUB = R // 128              # 8 u-panels of 128 rows
NVB = SLAB // 128           # 16 v-blocks per slab
M = 65                      # feature columns: hi32 | ones | lo32
PE_SLABS = 8                # slabs transposed on the PE instead of DMA XBAR
                            # (XBAR transposes serialize against other DMAs on
                            # this HW; PE-transpose measured ~3.6x faster)


# --- walrus compatibility: this container's walrus rejects >1 sync-wait per
# instruction, but Tile's scheduler can attach several (tail drain, DMAs with
# multiple producers, ...). Hoist extra waits onto single-wait NOPs inserted
# just before the instruction on the same engine stream.
def _split_multi_waits(nc):
    for bb in nc.m.functions[0].blocks:
        il = bb.instructions
        out = []
        changed = False
        for inst in il:
            si = inst.sync_info
            if si is not None and si.on_wait and len(si.on_wait) > 1:
                waits = list(si.on_wait)
                for i, w in enumerate(waits[:-1]):
                    nop = mybir.InstNoOp(name=f"{inst.name}-sw{i}",
                                         ins=[], outs=[])
                    nop.engine = inst.engine
                    nop.sync_info = mybir.SyncInfo(on_wait=[w], on_update=[])
                    nc.register_instruction(nop, overwrite=True)
                    out.append(nop)
                si.on_wait = waits[-1:]
                inst.sync_info = si
                changed = True
            out.append(inst)
        if changed:
            bb.instructions = out


def _emit_body(nc, tc, pools, adj, ufa_t, vfa_t, xu, xv, ident_s=None,
               pe_slabs=0):
    featp, edgep, edgeTp, adjp, stagep, vpsp, upsp, tpp = pools
    is_gt = mybir.AluOpType.is_gt

    upsum = upsp.tile([M, R], F32)          # persistent u-side accumulator

    for s in range(NSLAB):
        # load + edge build: adj int32 -> (cast-DMA) bf16 -> (is_gt 0) 0/1
        edge = edgep.tile([128, NUB, SLAB], BF16)
        for ub in range(NUB):
            adjf = adjp.tile([128, SLAB], BF16, tag="adjf")
            nc.gpsimd.dma_start(
                adjf[:], adj[ub * 128:(ub + 1) * 128, s * SLAB:(s + 1) * SLAB])
            nc.vector.tensor_scalar(
                edge[:, ub, :], adjf[:], 0.0, None, op0=is_gt)

        # v-side: X_v[:, slab] = sum_ub ufa[ub].T @ edge[ub]
        for h in range(2):                  # half-slab = 1024 v-cols
            vpsum = vpsp.tile([M, 1024], F32, tag="vpsum")
            for ub in range(NUB):
                for nb in range(2):
                    c0 = h * 1024 + nb * 512
                    nc.tensor.matmul(
                        vpsum[:, nb * 512:(nb + 1) * 512],
                        ufa_t[:, ub, :],
                        edge[:, ub, c0:c0 + 512],
                        start=(ub == 0), stop=(ub == NUB - 1))
            xv_s = stagep.tile([M, 1024], F32, tag="xv_s")
            nc.any.tensor_copy(xv_s[:], vpsum[:])
            nc.sync.dma_start(
                xv[:, s * SLAB + h * 1024: s * SLAB + (h + 1) * 1024], xv_s[:])

        # transpose the slab: edgeT[p, vb, u] = edge[u, vb*128 + p]
        # First `pe_slabs` slabs transpose on the PE (via identity matmul +
        # ACT copy-out), the rest on the DMA XBAR — balances the SDMA pool
        # against the tensor engine.
        edgeT = edgeTp.tile([128, NVB, R], BF16)
        if s < pe_slabs:
            for ub in range(NUB):
                for h in range(2):          # 1024-col chunk = 1 PSUM bank
                    tp = tpp.tile([128, NVB // 2, 128], BF16, tag="tp")
                    for vb in range(NVB // 2):
                        nc.tensor.transpose(
                            tp[:, vb, :],
                            edge[:, ub, h * 1024 + vb * 128:
                                 h * 1024 + (vb + 1) * 128],
                            ident_s[:])
                    nc.scalar.activation(
                        edgeT[:, h * (NVB // 2):(h + 1) * (NVB // 2),
                              ub * 128:(ub + 1) * 128],
                        tp[:],
                        mybir.ActivationFunctionType.Copy)
        else:
            for ub in range(NUB):
                nc.sync.dma_start(
                    out=edgeT[:, :, ub * 128:(ub + 1) * 128],
                    in_=edge[:, ub, :],
                    transpose=True)

        # u-side: X_u += sum_vb vfa[slab, vb].T @ edgeT[vb]
        for vb in range(NVB):
            for nb in range(R // 512):
                nc.tensor.matmul(
                    upsum[:, nb * 512:(nb + 1) * 512],
                    vfa_t[:, s * NVB + vb, :],
                    edgeT[:, vb, nb * 512:(nb + 1) * 512],
                    start=(s == 0 and vb == 0),
                    stop=(s == NSLAB - 1 and vb == NVB - 1))

    xu_s = stagep.tile([M, R], F32, tag="xu_s")
    nc.any.tensor_copy(xu_s[:], upsum[:])
    nc.sync.dma_start(xu[:], xu_s[:])


def build_nc(repeat: int = 1, pe_slabs: int = PE_SLABS):
    """One SPMD program: full inputs per core are its row shard of adj plus
    pre-tiled bf16 feature tensors; outputs are the feature-major partials."""
    nc = bass.Bass("TRN2", target_bir_lowering=False, debug=False)
    adj = nc.dram_tensor("adj", [R, N_V], mybir.dt.int32, kind="ExternalInput")
    # pre-tiled on host: [128, ntiles*M] with [p, a*M+m] = feat[a*128+p, m]
    ufa = nc.dram_tensor("ufa", [128, NUB * M], BF16, kind="ExternalInput")
    vfa = nc.dram_tensor("vfa", [128, (N_V // 128) * M], BF16,
                         kind="ExternalInput")
    ident = nc.dram_tensor("ident", [128, 128], BF16, kind="ExternalInput")
    xu = nc.dram_tensor("xu", [M, R], F32, kind="ExternalOutput")
    xv = nc.dram_tensor("xv", [M, N_V], F32, kind="ExternalOutput")

    with tile.TileContext(nc) as tc:
        with tc.tile_pool(name="feat", bufs=1) as featp, \
             tc.tile_pool(name="edge", bufs=2) as edgep, \
             tc.tile_pool(name="edgeT", bufs=2) as edgeTp, \
             tc.tile_pool(name="adj", bufs=3) as adjp, \
             tc.tile_pool(name="stage", bufs=2) as stagep, \
             tc.tile_pool(name="vps", bufs=2, space="PSUM") as vpsp, \
             tc.tile_pool(name="ups", bufs=1, space="PSUM") as upsp, \
             tc.tile_pool(name="tp", bufs=2, space="PSUM") as tpp:
            ufa_t = featp.tile([128, NUB, M], BF16)
            nc.sync.dma_start(ufa_t[:], ufa.rearrange("p (a m) -> p a m", m=M))
            vfa_t = featp.tile([128, N_V // 128, M], BF16)
            nc.sync.dma_start(vfa_t[:], vfa.rearrange("p (a m) -> p a m", m=M))
            ident_s = featp.tile([128, 128], BF16)
            nc.sync.dma_start(ident_s[:], ident[:])
            pools = (featp, edgep, edgeTp, adjp, stagep, vpsp, upsp, tpp)
            for _ in range(repeat):
                _emit_body(nc, tc, pools, adj, ufa_t, vfa_t, xu, xv,
                           ident_s=ident_s, pe_slabs=pe_slabs)
    _split_multi_waits(nc)
    return nc


def _tile_features(feat32: np.ndarray) -> np.ndarray:
    """[N,32] f32 -> pre-tiled [128, (N//128)*65] bf16 of [hi32|ones|lo32]."""
    n = feat32.shape[0]
    hi = feat32.astype(ml_dtypes.bfloat16)
    lo = (feat32 - hi.astype(np.float32)).astype(ml_dtypes.bfloat16)
    aug = np.zeros((n, M), dtype=ml_dtypes.bfloat16)
    aug[:, 0:32] = hi
    aug[:, 32] = 1.0
    aug[:, 33:65] = lo
    # [N, M] -> [ntiles, 128, M] -> [128, ntiles, M] -> [128, ntiles*M]
    return np.ascontiguousarray(
        aug.reshape(n // 128, 128, M).transpose(1, 0, 2).reshape(128, -1))


def _finalize(x: np.ndarray, w: np.ndarray) -> np.ndarray:
    """x [65, N] feature-major raw sums -> relu((num/deg) @ w) [N, 32]."""
    num = (x[0:32].astype(np.float64) + x[33:65].astype(np.float64)).T
    deg = x[32].astype(np.float64)
    agg = num / deg[:, None]
    return np.maximum(agg @ np.asarray(w, np.float64), 0.0).astype(np.float32)


_NC_CACHE: dict = {}


def _get_nc(repeat: int = 1, pe_slabs: int = PE_SLABS):
    key = (repeat, pe_slabs)
    if key not in _NC_CACHE:
        _NC_CACHE[key] = build_nc(repeat, pe_slabs)
    return _NC_CACHE[key]


def _device_inputs(adj, u_feature, v_feature):
    adj = np.asarray(adj)
    if adj.dtype != np.int32:
        adj = adj.astype(np.int32)
    vfa = _tile_features(np.asarray(v_feature, dtype=np.float32))
    ident = np.eye(128, dtype=ml_dtypes.bfloat16)
    uf = np.asarray(u_feature, dtype=np.float32)
    in_maps = []
    for c in range(N_CORES):
        in_maps.append({
            "adj": np.ascontiguousarray(adj[c * R:(c + 1) * R]),
            "ufa": _tile_features(uf[c * R:(c + 1) * R]),
            "vfa": vfa,
            "ident": ident,
        })
    return in_maps


def kernel(adj, u_feature, v_feature, weight_u, weight_v, _repeat: int = 1):
    in_maps = _device_inputs(adj, u_feature, v_feature)
    nc = _get_nc(_repeat)
    res = run_bass_kernel_spmd(nc, in_maps, core_ids=list(range(N_CORES)))
    xu = np.concatenate([res.results[c]["xu"] for c in range(N_CORES)], axis=1)
    xv = np.sum(np.stack([res.results[c]["xv"] for c in range(N_CORES)]),
                axis=0, dtype=np.float64)
    w_u = np.asarray(weight_u, dtype=np.float32)
    w_v = np.asarray(weight_v, dtype=np.float32)
    output_u = _finalize(xu, w_u)
    output_v = _finalize(xv, w_v)
    return output_u, output_v
